# revision 1
# baseline (speedup 1.0000x reference)
"""ArcFace combined-margin loss kernel for 8 TRN2 NeuronCores.

Strategy
--------
reference: cos = (f @ w.T) / (|f||w|); phi = arcface(cos);
outputs = s*(labels*phi + (1-labels)*cos); loss = mean over rows of
-(sum of log_softmax(outputs) at lab_pinds, masked) / L^2.

labels is the multi-hot of (lab_pinds, lengths), so outputs differs from
s*cos only at <=8 entries/row.  Device work is therefore:
  1. C-sharded (2500 classes/core, zero-padded to 2560) dense part: each
     core computes, for all 2048 rows, partial sums
     sexp[b] = sum_c exp(30*cos[b,c] - 30) over its class shard.
     Unit-normalized w rows and raw f rows are transposed on the
     TensorEngine (bf16), evicted from PSUM as fp8(e4m3) scaled by 16,
     and the dots run as fp8 DoubleRow matmuls (K=256 per instruction)
     into bank-aligned PSUM pairs.  PSUM is drained to an SBUF strip and
     one ACT exp per row-block applies scale 30/(256*|f_b|), bias -30,
     with a free-dim accumulate producing the row partial sums.
  2. B-sharded (256 rows/core) positive part: indirect-DMA gather of the
     2048 w rows addressed by lab_pinds, raw fp32 dots with f rows on DVE
     (the positives feed the loss directly, so they stay fp32).
  3. Per-row norm reciprocals (30/|f_b| and 1/|w_c|) as side outputs.
Host (numpy, float64) combines the tiny per-core partials: assembles
cos at positives, applies the arcface margin, corrects the denominator
(exp(30*phi)-exp(30*cos) at positives), logsumexp, masked ragged CE, mean.
No collectives are needed (the only cross-core reduction is over [2048]
scalars, done on host during unsharding).
"""

import math
import sys

import numpy as np

for _p in ("/opt/trn_rl_repo",):
    if _p not in sys.path:
        sys.path.append(_p)

import concourse.bass as bass
import concourse.bacc as bacc
import concourse.mybir as mybir
import concourse.tile as tile
from concourse.bass_utils import run_bass_kernel_spmd
from concourse.masks import make_identity
from contextlib import ExitStack

B, C, D, LMAX = 2048, 20000, 512, 8
NCORES = 8
CSH = C // NCORES          # 2500 real classes per core
CSHP = 2560                # padded to 5*512 (bank-aligned chunks)
BSH = B // NCORES          # 256 rows per core (positives shard)
NBLK = B // 128            # 16 row blocks
NW = 512                   # matmul N-chunk width (exactly one PSUM bank)
NCH = CSHP // NW           # 5 chunks per core
KC = D // 128              # 4 contraction chunks
CT = CSHP // 128           # 20 class tiles (all full)
S = 30.0
M_MARGIN = 0.5

F32 = mybir.dt.float32
BF16 = mybir.dt.bfloat16
FP8 = mybir.dt.float8e4
F8S = 16.0                 # fp8 pre-scale per operand (dots carry 256x)

_GRAPH = None


def build_graph():
    nc = bacc.Bacc()
    f_ext = nc.declare_dram_parameter("f", [B, D], F32, isOutput=False)
    wsh_ext = nc.declare_dram_parameter("wsh", [CSHP, D], F32, isOutput=False)
    w_ext = nc.declare_dram_parameter("w", [C, D], F32, isOutput=False)
    fsh_ext = nc.declare_dram_parameter("fsh", [BSH, D], F32, isOutput=False)
    pidx_ext = nc.declare_dram_parameter("pidx", [128, 16], mybir.dt.int32, isOutput=False)
    sexp_ext = nc.declare_dram_parameter("sexp", [128, NBLK * NCH], F32, isOutput=True)
    pdot_ext = nc.declare_dram_parameter("pdot", [128, 16], F32, isOutput=True)
    rf_ext = nc.declare_dram_parameter("rf30", [128, NBLK], F32, isOutput=True)
    rw_ext = nc.declare_dram_parameter("rwrec", [128, CT], F32, isOutput=True)

    mult = mybir.AluOpType.mult
    AF = mybir.ActivationFunctionType

    with ExitStack() as ctx:
        tc = ctx.enter_context(tile.TileContext(nc))
        const = ctx.enter_context(tc.tile_pool(name="const", bufs=1))
        resident = ctx.enter_context(tc.tile_pool(name="resident", bufs=1))
        fstage = ctx.enter_context(tc.tile_pool(name="fstage", bufs=4))
        wstage = ctx.enter_context(tc.tile_pool(name="wstage", bufs=4))
        wbfp = ctx.enter_context(tc.tile_pool(name="wbfp", bufs=3))
        scrp = ctx.enter_context(tc.tile_pool(name="scrp", bufs=3))
        esp = ctx.enter_context(tc.tile_pool(name="esp", bufs=4))
        ptr_pool = ctx.enter_context(tc.tile_pool(name="ptr", bufs=1, space="PSUM"))
        pmmA = ctx.enter_context(tc.tile_pool(name="pmmA", bufs=2, space="PSUM"))
        pmmB = ctx.enter_context(tc.tile_pool(name="pmmB", bufs=1, space="PSUM"))
        pmmC = ctx.enter_context(tc.tile_pool(name="pmmC", bufs=1, space="PSUM"))

        id_bf = const.tile([128, 128], BF16)
        make_identity(nc, id_bf[:])
        zbias = const.tile([128, 1], F32)
        nc.vector.memset(zbias[:], 0.0)
        nbias = const.tile([128, 1], F32)
        nc.vector.memset(nbias[:], -S)

        # resident tensors
        wT = resident.tile([128, KC, CSHP], FP8)      # normalized w, transposed
        fT = resident.tile([128, KC, B], FP8)        # raw f, transposed
        G = resident.tile([128, 16, D], F32)         # gathered positive w rows
        fsh_t = resident.tile([128, 2, D], F32)      # this core's f rows (raw)
        sexp_t = resident.tile([128, NBLK * NCH], F32)
        pdot_t = resident.tile([128, 16], F32)
        ss_f = resident.tile([128, NBLK], F32)
        tmp_f = resident.tile([128, NBLK], F32)
        rf30 = resident.tile([128, NBLK], F32)
        rf30s = resident.tile([128, NBLK], F32)
        ss_w = resident.tile([128, CT], F32)
        tmp_w = resident.tile([128, CT], F32)
        rw_rec = resident.tile([128, CT], F32)
        pidx_t = resident.tile([128, 16], mybir.dt.int32)

        # ---- positives gather (early: overlaps with everything) ----
        nc.sync.dma_start(pidx_t[:], pidx_ext[:, :])
        nc.sync.dma_start(fsh_t[:, 0, :], fsh_ext[0:128, :])
        nc.sync.dma_start(fsh_t[:, 1, :], fsh_ext[128:256, :])
        for s in range(16):
            nc.gpsimd.indirect_dma_start(
                out=G[:, s, :],
                out_offset=None,
                in_=w_ext[:, :],
                in_offset=bass.IndirectOffsetOnAxis(ap=pidx_t[:, s : s + 1], axis=0),
            )

        # ---- w path: row norms, scale to unit rows (bf16), transpose ----
        nc.vector.memset(rw_rec[:], 0.0)  # tail tile covers <128 partitions
        nc.vector.memset(sexp_t[:], 0.0)  # strip-exp fills col m*NCH only
        for ct in range(CT):
            r0 = ct * 128
            wt = wstage.tile([128, D], F32, tag="w")
            nc.sync.dma_start(wt[:, :], wsh_ext[r0 : r0 + 128, :])
            sc = scrp.tile([128, D], F32, tag="scr")
            nc.scalar.activation(
                sc[:], wt[:], AF.Square,
                bias=zbias[:], scale=1.0,
                accum_out=ss_w[:, ct : ct + 1],
            )
            # zero pad rows: keep sqrt/recip finite (wT pad cols end up 0)
            nc.vector.tensor_scalar_max(
                ss_w[:, ct : ct + 1], ss_w[:, ct : ct + 1], 1e-12
            )
            nc.scalar.activation(
                tmp_w[:, ct : ct + 1], ss_w[:, ct : ct + 1],
                AF.Sqrt, bias=zbias[:], scale=1.0,
            )
            nc.vector.reciprocal(
                rw_rec[:, ct : ct + 1], tmp_w[:, ct : ct + 1]
            )
            wbf = wbfp.tile([128, D], BF16, tag="wbf")
            nc.vector.tensor_scalar_mul(
                wbf[:, :], wt[:, :], rw_rec[:, ct : ct + 1]
            )
            pt = ptr_pool.tile([128, KC, 128], BF16, tag="ptr")
            for k in range(KC):
                nc.tensor.transpose(
                    pt[:, k, :], wbf[:, k * 128 : (k + 1) * 128], id_bf[:]
                )
            nc.vector.tensor_scalar_mul(
                wT[:, :, r0 : r0 + 128], pt[:], F8S
            )
        nc.sync.dma_start(rw_ext[:, :], rw_rec[:])

        # ---- f path: row norms (for ACT scale), raw transpose ----
        for m in range(NBLK):
            ft = fstage.tile([128, D], F32, tag="f")
            nc.sync.dma_start(ft[:], f_ext[m * 128 : (m + 1) * 128, :])
            sc = scrp.tile([128, D], F32, tag="scr")
            nc.scalar.activation(
                sc[:], ft[:], AF.Square,
                bias=zbias[:], scale=1.0,
                accum_out=ss_f[:, m : m + 1],
            )
            fb = wbfp.tile([128, D], BF16, tag="fb")
            nc.vector.tensor_copy(out=fb[:], in_=ft[:])
            pt = ptr_pool.tile([128, KC, 128], BF16, tag="ptr")
            for k in range(KC):
                nc.tensor.transpose(
                    pt[:, k, :], fb[:, k * 128 : (k + 1) * 128], id_bf[:]
                )
            nc.vector.tensor_scalar_mul(
                fT[:, :, m * 128 : (m + 1) * 128], pt[:], F8S
            )
        # rf30 = 30 / |f|  (sqrt(ss/900) then reciprocal)
        nc.scalar.activation(
            tmp_f[:], ss_f[:], AF.Sqrt, bias=zbias[:], scale=1.0 / (S * S)
        )
        nc.vector.reciprocal(rf30[:], tmp_f[:])
        nc.vector.tensor_scalar_mul(rf30s[:], rf30[:], 1.0 / (F8S * F8S))
        nc.sync.dma_start(rf_ext[:, :], rf30[:])


        # ---- positive dots: pdot[p, j+8h] = f[h*128+p] . G[p, j+8h] ----
        for h in range(2):
            for j in range(LMAX):
                s = j + LMAX * h
                sc = scrp.tile([128, D], F32, tag="scr")
                nc.vector.scalar_tensor_tensor(
                    out=sc[:], in0=G[:, s, :], scalar=1.0, in1=fsh_t[:, h, :],
                    op0=mult, op1=mult,
                    accum_out=pdot_t[:, s : s + 1],
                )
        nc.sync.dma_start(pdot_ext[:, :], pdot_t[:])

        # ---- main loop: dots -> exp(30*cos - 30) -> per-row accumulate ----
        for m in range(NBLK):
            pA = pmmA.tile([128, 2, NW], F32, tag="mmA", name=f"mmA_{m}")
            pB = pmmB.tile([128, 2, NW], F32, tag="mmB", name=f"mmB_{m}")
            pC = pmmC.tile([128, NW], F32, tag="mmC", name=f"mmC_{m}")
            # single-buffered B and C run first so their evicts have a full
            # block of slack before block m+1 reuses the banks; A (double
            # buffered) never stalls.
            ps = [pB[:, 0, :], pB[:, 1, :], pC[:], pA[:, 0, :], pA[:, 1, :]]
            for k2 in range(KC // 2):
                for i, n in enumerate((2, 3, 4, 0, 1)):
                    nc.tensor.matmul(
                        ps[i],
                        fT[:, 2 * k2 : 2 * k2 + 2, m * 128 : (m + 1) * 128],
                        wT[:, 2 * k2 : 2 * k2 + 2, n * NW : (n + 1) * NW],
                        start=(k2 == 0),
                        stop=(k2 == KC // 2 - 1),
                        perf_mode=mybir.MatmulPerfMode.DoubleRow,
                    )
            strip = esp.tile([128, NCH * NW], F32, tag="es", name=f"st{m}")
            nc.any.tensor_copy(out=strip[:, 2 * NW : 4 * NW], in_=pB[:])
            nc.any.tensor_copy(out=strip[:, 4 * NW : 5 * NW], in_=pC[:])
            nc.any.tensor_copy(out=strip[:, 0 : 2 * NW], in_=pA[:])
            edump = esp.tile([128, NCH * NW], BF16, tag="ed", name=f"ed{m}")
            nc.scalar.activation(
                edump[:], strip[:], AF.Exp,
                bias=nbias[:], scale=rf30s[:, m : m + 1],
                accum_out=sexp_t[:, m * NCH : m * NCH + 1],
            )
        nc.sync.dma_start(sexp_ext[:, :], sexp_t[:])


    nc.finalize()
    return nc


def _get_graph():
    global _GRAPH
    if _GRAPH is None:
        _GRAPH = build_graph()
    return _GRAPH


def make_in_maps(f, lab_word2vec, lab_pinds):
    f = np.ascontiguousarray(np.asarray(f, dtype=np.float32))
    w = np.ascontiguousarray(np.asarray(lab_word2vec, dtype=np.float32))
    pinds = np.asarray(lab_pinds, dtype=np.int64)
    in_maps = []
    for i in range(NCORES):
        # slot s = j + 8h at partition p  <-  lab_pinds[i*256 + h*128 + p, j]
        pidx = np.zeros((128, 16), dtype=np.int32)
        for h in range(2):
            for j in range(LMAX):
                pidx[:, j + LMAX * h] = pinds[
                    i * BSH + h * 128 : i * BSH + h * 128 + 128, j
                ]
        wsh = np.zeros((CSHP, D), dtype=np.float32)
        wsh[:CSH] = w[i * CSH : (i + 1) * CSH]
        in_maps.append(
            {
                "f": f,
                "wsh": wsh,
                "w": w,
                "fsh": np.ascontiguousarray(f[i * BSH : (i + 1) * BSH]),
                "pidx": pidx,
            }
        )
    return in_maps


def combine(outs, lab_pinds, lengths):
    """outs: list of 8 dicts with sexp/pdot/rf30/rwrec. Returns float32 loss."""
    pinds = np.asarray(lab_pinds, dtype=np.int64)
    lens = np.asarray(lengths, dtype=np.int64)

    # S_shift[b] = sum_c exp(30 cos - 30)
    s_shift = np.zeros(B, dtype=np.float64)
    for i in range(NCORES):
        se = outs[i]["sexp"].astype(np.float64)  # [128, NBLK*NCH]
        per_block = se.reshape(128, NBLK, NCH).sum(axis=2)  # [128, NBLK]
        s_shift += per_block.T.reshape(B)  # b = m*128 + p
    # the 60 zero-pad classes per core contribute exp(-30) each (cos = 0)
    s_shift -= NCORES * (CSHP - CSH) * math.exp(-S)

    rf = outs[0]["rf30"].astype(np.float64).T.reshape(B) / S  # 1/|f_b|

    rw = np.zeros(C, dtype=np.float64)
    for i in range(NCORES):
        rr = outs[i]["rwrec"].astype(np.float64)  # [128, CT]
        rw[i * CSH : (i + 1) * CSH] = rr.T.reshape(CSHP)[:CSH]

    # positive raw dots -> [B, LMAX]
    pdot = np.zeros((B, LMAX), dtype=np.float64)
    for i in range(NCORES):
        pd = outs[i]["pdot"].astype(np.float64)  # [128, 16]
        for h in range(2):
            for j in range(LMAX):
                pdot[i * BSH + h * 128 : i * BSH + h * 128 + 128, j] = pd[
                    :, j + LMAX * h
                ]

    cos = pdot * rf[:, None] * rw[pinds]  # [B, LMAX]
    cos_m, sin_m = math.cos(M_MARGIN), math.sin(M_MARGIN)
    th = math.cos(math.pi - M_MARGIN)
    mm = math.sin(math.pi - M_MARGIN) * M_MARGIN
    sine = np.sqrt(np.clip(1.0 - cos * cos, 0.0, 1.0))
    phi = cos * cos_m - sine * sin_m
    phi = np.where(cos > th, phi, cos - mm)

    mask = (np.arange(LMAX)[None, :] < lens[:, None]).astype(np.float64)
    corr = (mask * (np.exp(S * phi - S) - np.exp(S * cos - S))).sum(axis=1)
    z = S + np.log(s_shift + corr)  # logsumexp of outputs, [B]
    pos_sum = (mask * (S * phi)).sum(axis=1)
    L = lens.astype(np.float64)
    per_sample = (L * z - pos_sum) / (L * L)
    return np.float32(per_sample.mean())


def kernel(f, labels, lab_word2vec, lab_pinds, lengths):
    nc = _get_graph()
    in_maps = make_in_maps(f, lab_word2vec, lab_pinds)
    res = run_bass_kernel_spmd(nc, in_maps, core_ids=list(range(NCORES)))
    return combine(res.results, lab_pinds, lengths)



# revision 3
# speedup vs baseline: 1.7697x; 1.7697x over previous
"""ArcFace combined-margin loss kernel for 8 TRN2 NeuronCores.

Strategy
--------
reference: cos = (f @ w.T) / (|f||w|); phi = arcface(cos);
outputs = s*(labels*phi + (1-labels)*cos); loss = mean over rows of
-(sum of log_softmax(outputs) at lab_pinds, masked) / L^2.

labels is the multi-hot of (lab_pinds, lengths), so outputs differs from
s*cos only at <=8 entries/row.  The only O(B*C) work is the dense
sexp[b] = sum_c exp(30*cos[b,c] - 30); everything else is O(B*LMAX) or
O((B+C)*D) and runs on host in float64.

Device (per core, classes C-sharded 2500/core zero-padded to 2560):
  inputs are pre-normalized, pre-transposed fp8(e4m3, x16) operands
  prepared on host.  Main loop over 16 row-blocks x 5 class-chunks:
  fp8 DoubleRow matmuls (K=256/instr) accumulate dots into PSUM, and
  the ScalarE reads each PSUM bank directly with one Exp activation
  (scale 30/256, bias -30) whose accum_out produces the per-row
  partial sum.  Output is just sexp [128, 80] per core.

Host (numpy, float64): row norms of f and w, normalization + transpose
+ fp8 quantization of the matmul operands, exact positive-class cos
via gather, arcface margin, denominator correction (dedup'd), ragged
CE, mean.  No collectives (cross-core reduction of [2048] scalars
happens on host during unsharding).
"""

import math
import sys

import numpy as np
from ml_dtypes import float8_e4m3

for _p in ("/opt/trn_rl_repo",):
    if _p not in sys.path:
        sys.path.append(_p)

import concourse.bass as bass
import concourse.bacc as bacc
import concourse.mybir as mybir
import concourse.tile as tile
from concourse.bass_utils import run_bass_kernel_spmd
from contextlib import ExitStack

B, C, D, LMAX = 2048, 20000, 512, 8
NCORES = 8
CSH = C // NCORES          # 2500 real classes per core
CSHP = 2560                # padded to 5*512 (bank-aligned chunks)
NBLK = B // 128            # 16 row blocks
NW = 512                   # matmul N-chunk width (exactly one PSUM bank)
NCH = CSHP // NW           # 5 chunks per core
KC = D // 128              # 4 contraction chunks
NFP = 4                    # f pieces (4 row-blocks each)
S = 30.0
M_MARGIN = 0.5

F32 = mybir.dt.float32
BF16 = mybir.dt.bfloat16
FP8 = mybir.dt.float8e4
F8S = 16.0                 # fp8 pre-scale per operand (dots carry 256x)

_GRAPH = None


def build_graph():
    nc = bacc.Bacc()
    ft_ext = [
        nc.declare_dram_parameter(f"ft8_{q}", [128, KC, NW], FP8, isOutput=False)
        for q in range(NFP)
    ]
    wt_ext = [
        nc.declare_dram_parameter(f"wt8_{n}", [128, KC, NW], FP8, isOutput=False)
        for n in range(NCH)
    ]
    sexp_ext = nc.declare_dram_parameter("sexp", [128, NBLK * NCH], F32, isOutput=True)

    AF = mybir.ActivationFunctionType

    with ExitStack() as ctx:
        tc = ctx.enter_context(tile.TileContext(nc))
        const = ctx.enter_context(tc.tile_pool(name="const", bufs=1))
        resident = ctx.enter_context(tc.tile_pool(name="resident", bufs=1))
        esp = ctx.enter_context(tc.tile_pool(name="esp", bufs=4))
        pmm = ctx.enter_context(tc.tile_pool(name="pmm", bufs=8, space="PSUM"))

        nbias = const.tile([128, 1], F32)
        nc.vector.memset(nbias[:], -S)

        fT = [resident.tile([128, KC, NW], FP8, name=f"fT{q}") for q in range(NFP)]
        wT = [resident.tile([128, KC, NW], FP8, name=f"wT{n}") for n in range(NCH)]
        sexp_t = resident.tile([128, NBLK * NCH], F32)

        for q in range(NFP):
            nc.sync.dma_start(fT[q][:], ft_ext[q][:, :, :])
        for n in range(NCH):
            nc.sync.dma_start(wT[n][:], wt_ext[n][:, :, :])

        # main loop: dots -> exp(30*cos - 30) -> per-row accumulate
        for m in range(NBLK):
            q, r = divmod(m, NFP)
            ps = [
                pmm.tile([128, NW], F32, tag="mm", name=f"ps_{m}_{n}")
                for n in range(NCH)
            ]
            for k2 in range(KC // 2):
                for n in range(NCH):
                    nc.tensor.matmul(
                        ps[n][:],
                        fT[q][:, 2 * k2 : 2 * k2 + 2, r * 128 : (r + 1) * 128],
                        wT[n][:, 2 * k2 : 2 * k2 + 2, :],
                        start=(k2 == 0),
                        stop=(k2 == KC // 2 - 1),
                        perf_mode=mybir.MatmulPerfMode.DoubleRow,
                    )
            for n in range(NCH):
                ed = esp.tile([128, NW], BF16, tag="ed", name=f"ed_{m}_{n}")
                t = NCH * m + n
                nc.scalar.activation(
                    ed[:], ps[n][:], AF.Exp,
                    bias=nbias[:], scale=S / (F8S * F8S),
                    accum_out=sexp_t[:, t : t + 1],
                )
        nc.sync.dma_start(sexp_ext[:, :], sexp_t[:])

    nc.finalize()
    return nc


def _get_graph():
    global _GRAPH
    if _GRAPH is None:
        _GRAPH = build_graph()
    return _GRAPH


def _to_kpn(xT):
    """[D, N] (d-major) -> [128, KC, N] with partition p = d % 128, k = d // 128."""
    Dd, N = xT.shape
    return np.ascontiguousarray(xT.reshape(KC, 128, N).transpose(1, 0, 2))


def make_in_maps(f, lab_word2vec, lab_pinds=None):
    f = np.asarray(f, dtype=np.float64)
    w = np.asarray(lab_word2vec, dtype=np.float64)
    fn = np.linalg.norm(f, axis=1)
    wn = np.linalg.norm(w, axis=1)
    fhatT = (F8S * (f / fn[:, None]).T).astype(np.float32)   # [D, B]
    ft_kpn = _to_kpn(fhatT).astype(float8_e4m3)              # [128, KC, B]
    ft_pieces = [
        np.ascontiguousarray(ft_kpn[:, :, q * NW : (q + 1) * NW]) for q in range(NFP)
    ]
    in_maps = []
    for i in range(NCORES):
        wpad = np.zeros((CSHP, D), dtype=np.float64)
        wsh = w[i * CSH : (i + 1) * CSH]
        wpad[:CSH] = wsh / wn[i * CSH : (i + 1) * CSH, None]
        wt_kpn = _to_kpn((F8S * wpad.T).astype(np.float32)).astype(float8_e4m3)
        m = {f"ft8_{q}": ft_pieces[q] for q in range(NFP)}
        for n in range(NCH):
            m[f"wt8_{n}"] = np.ascontiguousarray(wt_kpn[:, :, n * NW : (n + 1) * NW])
        in_maps.append(m)
    return in_maps


def host_finish(outs, f, lab_word2vec, lab_pinds, lengths):
    """outs: list of 8 dicts with sexp. Returns float32 loss."""
    f = np.asarray(f, dtype=np.float64)
    w = np.asarray(lab_word2vec, dtype=np.float64)
    pinds = np.asarray(lab_pinds, dtype=np.int64)
    lens = np.asarray(lengths, dtype=np.int64)

    # S_shift[b] = sum_c exp(30 cos - 30)
    s_shift = np.zeros(B, dtype=np.float64)
    for i in range(NCORES):
        se = outs[i]["sexp"].astype(np.float64)          # [128, NBLK*NCH]
        per_block = se.reshape(128, NBLK, NCH).sum(axis=2)  # [128, NBLK]
        s_shift += per_block.T.reshape(B)                # b = m*128 + p
    # the 60 zero-pad classes per core contribute exp(-30) each (dot = 0)
    s_shift -= NCORES * (CSHP - CSH) * math.exp(-S)

    # exact positive-class cosines on host
    fn = np.linalg.norm(f, axis=1)                       # [B]
    wn = np.linalg.norm(w, axis=1)                       # [C]
    wsel = w[pinds]                                      # [B, LMAX, D]
    dots = np.einsum("bd,bld->bl", f, wsel)              # [B, LMAX]
    cos = dots / np.maximum(fn[:, None] * wn[pinds], 1e-8)

    cos_m, sin_m = math.cos(M_MARGIN), math.sin(M_MARGIN)
    th = math.cos(math.pi - M_MARGIN)
    mm = math.sin(math.pi - M_MARGIN) * M_MARGIN
    sine = np.sqrt(np.clip(1.0 - cos * cos, 0.0, 1.0))
    phi = cos * cos_m - sine * sin_m
    phi = np.where(cos > th, phi, cos - mm)

    mask = (np.arange(LMAX)[None, :] < lens[:, None])    # [B, LMAX] bool
    # dedup: a class replaced once in the denominator even if in 2 slots
    dup = np.zeros_like(mask)
    for j in range(1, LMAX):
        for j2 in range(j):
            dup[:, j] |= mask[:, j2] & (pinds[:, j2] == pinds[:, j])
    uniq = mask & ~dup
    corr = (uniq * (np.exp(S * phi - S) - np.exp(S * cos - S))).sum(axis=1)
    z = S + np.log(s_shift + corr)                       # logsumexp, [B]
    pos_sum = (mask * (S * phi)).sum(axis=1)
    L = lens.astype(np.float64)
    per_sample = (L * z - pos_sum) / (L * L)
    return np.float32(per_sample.mean())


def kernel(f, labels, lab_word2vec, lab_pinds, lengths):
    nc = _get_graph()
    in_maps = make_in_maps(f, lab_word2vec)
    res = run_bass_kernel_spmd(nc, in_maps, core_ids=list(range(NCORES)))
    return host_finish(res.results, f, lab_word2vec, lab_pinds, lengths)


# revision 14
# speedup vs baseline: 2.0536x; 1.1605x over previous
"""ArcFace combined-margin loss kernel for 8 TRN2 NeuronCores.

Strategy
--------
reference: cos = (f @ w.T) / (|f||w|); phi = arcface(cos);
outputs = s*(labels*phi + (1-labels)*cos); loss = mean over rows of
-(sum of log_softmax(outputs) at lab_pinds, masked) / L^2.

labels is the multi-hot of (lab_pinds, lengths), so outputs differs from
s*cos only at <=8 entries/row.  The only O(B*C) work is the dense
sexp[b] = sum_c exp(30*cos[b,c] - 30); everything else is O(B*LMAX) or
O((B+C)*D) and runs on host in float64.

Device (per core, classes C-sharded 2500/core zero-padded to 2560):
  inputs are pre-normalized, pre-transposed fp8(e4m3, x16) operands
  prepared on host.  Main loop over 16 row-blocks x 5 class-chunks:
  fp8 DoubleRow matmuls (K=256/instr) accumulate dots into PSUM, and
  the ScalarE reads each PSUM bank directly with one Exp activation
  (scale 30/256, bias -30) whose accum_out produces the per-row
  partial sum.  Output is just sexp [128, 80] per core.

Host (numpy, float64): row norms of f and w, normalization + transpose
+ fp8 quantization of the matmul operands, exact positive-class cos
via gather, arcface margin, denominator correction (dedup'd), ragged
CE, mean.  No collectives (cross-core reduction of [2048] scalars
happens on host during unsharding).
"""

import math
import sys

import numpy as np
from ml_dtypes import float8_e4m3

for _p in ("/opt/trn_rl_repo",):
    if _p not in sys.path:
        sys.path.append(_p)

import concourse.bass as bass
import concourse.bacc as bacc
import concourse.mybir as mybir
import concourse.tile as tile
from concourse.bass_utils import run_bass_kernel_spmd
from contextlib import ExitStack

B, C, D, LMAX = 2048, 20000, 512, 8
NCORES = 8
CSH = C // NCORES          # 2500 real classes per core
CSHP = 2560                # padded to 5*512 (bank-aligned chunks)
NBLK = B // 128            # 16 row blocks
NW = 512                   # matmul N-chunk width (one fp32 PSUM bank)
NCH = CSHP // NW           # 5 chunks per block per core
NCHT = NBLK * NCH          # 80 chunks streamed per core
GRP = 4                    # chunks per PSUM tile (one ACT instr each)
NGRP = NCHT // GRP         # 20 PSUM tiles / ACT instructions
KC = D // 128              # 4 contraction chunks
NFP = 4                    # f pieces (4 row-blocks each)
S = 30.0
M_MARGIN = 0.5

F32 = mybir.dt.float32
BF16 = mybir.dt.bfloat16
FP8 = mybir.dt.float8e4
F8S = 16.0                 # fp8 pre-scale per operand (dots carry 256x)

_GRAPH = None


def build_graph():
    nc = bacc.Bacc()
    ft_ext = [
        nc.declare_dram_parameter(f"ft8_{q}", [128, KC, NW], FP8, isOutput=False)
        for q in range(NFP)
    ]
    wt_ext = [
        nc.declare_dram_parameter(f"wt8_{n}", [128, KC, NW], FP8, isOutput=False)
        for n in range(NCH)
    ]
    parts_ext = nc.declare_dram_parameter("parts", [128, 2 * NGRP], F32, isOutput=True)

    AF = mybir.ActivationFunctionType

    with ExitStack() as ctx:
        tc = ctx.enter_context(tile.TileContext(nc))
        const = ctx.enter_context(tc.tile_pool(name="const", bufs=1))
        resident = ctx.enter_context(tc.tile_pool(name="resident", bufs=1))
        esp = ctx.enter_context(tc.tile_pool(name="esp", bufs=3))
        pmm = ctx.enter_context(tc.tile_pool(name="pmm", bufs=2, space="PSUM"))

        nbias = const.tile([128, 1], F32)
        nc.vector.memset(nbias[:], -S)

        fT = [resident.tile([128, KC, NW], FP8, name=f"fT{q}") for q in range(NFP)]
        wT = [resident.tile([128, KC, NW], FP8, name=f"wT{n}") for n in range(NCH)]
        parts_t = resident.tile([128, 2 * NGRP], F32)
        nc.vector.memset(parts_t[:], 0.0)

        for q in range(NFP):
            nc.sync.dma_start(fT[q][:], ft_ext[q][:, :, :])
        for n in range(NCH):
            nc.sync.dma_start(wT[n][:], wt_ext[n][:, :, :])

        # stream of 80 chunks (block-major) through 4-bank PSUM tiles:
        # dots -> one exp sweep per tile -> per-block DVE partial reduces
        for j in range(NGRP):
            ps = pmm.tile([128, GRP, NW], F32, tag="mm", name=f"ps_{j}")
            for s in range(GRP):
                c = GRP * j + s               # global chunk index
                m, n = divmod(c, NCH)         # row block / class chunk
                q, r = divmod(m, NFP)
                for k2 in range(KC // 2):
                    nc.tensor.matmul(
                        ps[:, s, :],
                        fT[q][:, 2 * k2 : 2 * k2 + 2, r * 128 : (r + 1) * 128],
                        wT[n][:, 2 * k2 : 2 * k2 + 2, :],
                        start=(k2 == 0),
                        stop=(k2 == KC // 2 - 1),
                        perf_mode=mybir.MatmulPerfMode.DoubleRow,
                    )
            ed = esp.tile([128, GRP * NW], BF16, tag="ed", name=f"ed_{j}")
            nc.scalar.activation(
                ed[:], ps[:], AF.Exp,
                bias=nbias[:], scale=S / (F8S * F8S),
            )
            # slots grouped by row block (a tile spans at most 2 blocks)
            b0 = (GRP * j) // NCH
            b1 = (GRP * j + GRP - 1) // NCH
            for bi, b in enumerate(range(b0, b1 + 1)):
                sl = [s for s in range(GRP) if (GRP * j + s) // NCH == b]
                s0, s1 = sl[0], sl[-1] + 1
                nc.vector.tensor_reduce(
                    parts_t[:, 2 * j + bi : 2 * j + bi + 1],
                    ed[:, s0 * NW : s1 * NW],
                    axis=mybir.AxisListType.X, op=mybir.AluOpType.add,
                )
        nc.sync.dma_start(parts_ext[:, :], parts_t[:])

    nc.finalize()
    return nc


def _get_graph():
    global _GRAPH
    if _GRAPH is None:
        _GRAPH = build_graph()
    return _GRAPH


def _to_kpn(xT):
    """[D, N] (d-major) -> [128, KC, N] with partition p = d % 128, k = d // 128."""
    Dd, N = xT.shape
    return np.ascontiguousarray(xT.reshape(KC, 128, N).transpose(1, 0, 2))


def make_in_maps(f, lab_word2vec, lab_pinds=None):
    f = np.asarray(f, dtype=np.float64)
    w = np.asarray(lab_word2vec, dtype=np.float64)
    fn = np.linalg.norm(f, axis=1)
    wn = np.linalg.norm(w, axis=1)
    fhatT = (F8S * (f / fn[:, None]).T).astype(np.float32)   # [D, B]
    ft_kpn = _to_kpn(fhatT).astype(float8_e4m3)              # [128, KC, B]
    ft_pieces = [
        np.ascontiguousarray(ft_kpn[:, :, q * NW : (q + 1) * NW]) for q in range(NFP)
    ]
    in_maps = []
    for i in range(NCORES):
        wpad = np.zeros((CSHP, D), dtype=np.float64)
        wsh = w[i * CSH : (i + 1) * CSH]
        wpad[:CSH] = wsh / wn[i * CSH : (i + 1) * CSH, None]
        wt_kpn = _to_kpn((F8S * wpad.T).astype(np.float32)).astype(float8_e4m3)
        m = {f"ft8_{q}": ft_pieces[q] for q in range(NFP)}
        for n in range(NCH):
            m[f"wt8_{n}"] = np.ascontiguousarray(wt_kpn[:, :, n * NW : (n + 1) * NW])
        in_maps.append(m)
    return in_maps


def host_finish(outs, f, lab_word2vec, lab_pinds, lengths):
    """outs: list of 8 dicts with sexp. Returns float32 loss."""
    f = np.asarray(f, dtype=np.float64)
    w = np.asarray(lab_word2vec, dtype=np.float64)
    pinds = np.asarray(lab_pinds, dtype=np.int64)
    lens = np.asarray(lengths, dtype=np.int64)

    # S_shift[b] = sum_c exp(30 cos - 30)
    # parts[:, 2j+bi] is the partial sum of exp over the slots of PSUM
    # tile j that fall in row block b0(j)+bi; fold them back per block.
    blk_of_col = np.zeros(2 * NGRP, dtype=np.int64)
    col_used = np.zeros(2 * NGRP, dtype=bool)
    for j in range(NGRP):
        b0 = (GRP * j) // NCH
        b1 = (GRP * j + GRP - 1) // NCH
        for bi, b in enumerate(range(b0, b1 + 1)):
            blk_of_col[2 * j + bi] = b
            col_used[2 * j + bi] = True
    s_shift = np.zeros(B, dtype=np.float64)
    for i in range(NCORES):
        pa = outs[i]["parts"].astype(np.float64)         # [128, 2*NGRP]
        per_block = np.zeros((128, NBLK), dtype=np.float64)
        for col in range(2 * NGRP):
            if col_used[col]:
                per_block[:, blk_of_col[col]] += pa[:, col]
        s_shift += per_block.T.reshape(B)                # b = m*128 + p
    # the 60 zero-pad classes per core contribute exp(-30) each (dot = 0)
    s_shift -= NCORES * (CSHP - CSH) * math.exp(-S)

    # exact positive-class cosines on host
    fn = np.linalg.norm(f, axis=1)                       # [B]
    wn = np.linalg.norm(w, axis=1)                       # [C]
    wsel = w[pinds]                                      # [B, LMAX, D]
    dots = np.einsum("bd,bld->bl", f, wsel)              # [B, LMAX]
    cos = dots / np.maximum(fn[:, None] * wn[pinds], 1e-8)

    cos_m, sin_m = math.cos(M_MARGIN), math.sin(M_MARGIN)
    th = math.cos(math.pi - M_MARGIN)
    mm = math.sin(math.pi - M_MARGIN) * M_MARGIN
    sine = np.sqrt(np.clip(1.0 - cos * cos, 0.0, 1.0))
    phi = cos * cos_m - sine * sin_m
    phi = np.where(cos > th, phi, cos - mm)

    mask = (np.arange(LMAX)[None, :] < lens[:, None])    # [B, LMAX] bool
    # dedup: a class replaced once in the denominator even if in 2 slots
    dup = np.zeros_like(mask)
    for j in range(1, LMAX):
        for j2 in range(j):
            dup[:, j] |= mask[:, j2] & (pinds[:, j2] == pinds[:, j])
    uniq = mask & ~dup
    corr = (uniq * (np.exp(S * phi - S) - np.exp(S * cos - S))).sum(axis=1)
    z = S + np.log(s_shift + corr)                       # logsumexp, [B]
    pos_sum = (mask * (S * phi)).sum(axis=1)
    L = lens.astype(np.float64)
    per_sample = (L * z - pos_sum) / (L * L)
    return np.float32(per_sample.mean())


def kernel(f, labels, lab_word2vec, lab_pinds, lengths):
    nc = _get_graph()
    in_maps = make_in_maps(f, lab_word2vec)
    res = run_bass_kernel_spmd(nc, in_maps, core_ids=list(range(NCORES)))
    return host_finish(res.results, f, lab_word2vec, lab_pinds, lengths)


# revision 20
# speedup vs baseline: 2.0991x; 1.0222x over previous
"""ArcFace combined-margin loss kernel for 8 TRN2 NeuronCores.

Strategy
--------
reference: cos = (f @ w.T) / (|f||w|); phi = arcface(cos);
outputs = s*(labels*phi + (1-labels)*cos); loss = mean over rows of
-(sum of log_softmax(outputs) at lab_pinds, masked) / L^2.

labels is the multi-hot of (lab_pinds, lengths), so outputs differs from
s*cos only at <=8 entries/row.  The only O(B*C) work is the dense
sexp[b] = sum_c exp(30*cos[b,c] - 30); everything else is O(B*LMAX) or
O((B+C)*D) and runs on host in float64.

Device (per core, classes C-sharded 2500/core zero-padded to 2560):
  inputs are pre-normalized, pre-transposed fp8(e4m3, x16) operands
  prepared on host.  Main loop over 16 row-blocks x 5 class-chunks:
  fp8 DoubleRow matmuls (K=256/instr) accumulate dots into PSUM, and
  the ScalarE reads each PSUM bank directly with one Exp activation
  (scale 30/256, bias -30) whose accum_out produces the per-row
  partial sum.  Output is just sexp [128, 80] per core.

Host (numpy, float64): row norms of f and w, normalization + transpose
+ fp8 quantization of the matmul operands, exact positive-class cos
via gather, arcface margin, denominator correction (dedup'd), ragged
CE, mean.  No collectives (cross-core reduction of [2048] scalars
happens on host during unsharding).
"""

import math
import sys

import numpy as np
from ml_dtypes import float8_e4m3

for _p in ("/opt/trn_rl_repo",):
    if _p not in sys.path:
        sys.path.append(_p)

import concourse.bass as bass
import concourse.bacc as bacc
import concourse.mybir as mybir
import concourse.tile as tile
from concourse.bass_utils import run_bass_kernel_spmd
from contextlib import ExitStack

B, C, D, LMAX = 2048, 20000, 512, 8
NCORES = 8
CSH = C // NCORES          # 2500 real classes per core
CSHP = 2560                # padded to 5*512 (bank-aligned chunks)
NBLK = B // 128            # 16 row blocks
NW = 512                   # matmul N-chunk width (one fp32 PSUM bank)
NCH = CSHP // NW           # 5 chunks per block per core
NCHT = NBLK * NCH          # 80 chunks streamed per core
GRP = 4                    # chunks per PSUM tile (one ACT instr each)
NGRP = NCHT // GRP         # 20 PSUM tiles / ACT instructions
KC = D // 128              # 4 contraction chunks
NFP = 4                    # f pieces (4 row-blocks each)
S = 30.0
M_MARGIN = 0.5

F32 = mybir.dt.float32
BF16 = mybir.dt.bfloat16
FP8 = mybir.dt.float8e4
F8S = 16.0                 # fp8 pre-scale per operand (dots carry 256x)

_GRAPH = None


def build_graph():
    nc = bacc.Bacc()
    ft_ext = [
        nc.declare_dram_parameter(f"ft8_{q}", [128, KC, NW], FP8, isOutput=False)
        for q in range(NFP)
    ]
    wt_ext = [
        nc.declare_dram_parameter(f"wt8_{n}", [128, KC, NW], FP8, isOutput=False)
        for n in range(NCH)
    ]
    parts_ext = nc.declare_dram_parameter("parts", [128, 2 * NGRP], F32, isOutput=True)

    AF = mybir.ActivationFunctionType

    with ExitStack() as ctx:
        tc = ctx.enter_context(tile.TileContext(nc))
        const = ctx.enter_context(tc.tile_pool(name="const", bufs=1))
        resident = ctx.enter_context(tc.tile_pool(name="resident", bufs=1))
        esp = ctx.enter_context(tc.tile_pool(name="esp", bufs=3))
        dummy = ctx.enter_context(tc.tile_pool(name="dummy", bufs=2))
        pmm = ctx.enter_context(tc.tile_pool(name="pmm", bufs=2, space="PSUM"))

        nbias = const.tile([128, 1], F32)
        nc.vector.memset(nbias[:], -S)

        fT = [resident.tile([128, KC, NW], FP8, name=f"fT{q}") for q in range(NFP)]
        wT = [resident.tile([128, KC, NW], FP8, name=f"wT{n}") for n in range(NCH)]
        parts_t = resident.tile([128, 2 * NGRP], F32)
        nc.vector.memset(parts_t[:], 0.0)

        # spread input DMA descriptor writes over all engine queues, in the
        # order the matmul stream consumes the pieces
        loads = [
            (fT[0], ft_ext[0]), (wT[0], wt_ext[0]), (wT[1], wt_ext[1]),
            (wT[2], wt_ext[2]), (wT[3], wt_ext[3]), (wT[4], wt_ext[4]),
            (fT[1], ft_ext[1]), (fT[2], ft_ext[2]), (fT[3], ft_ext[3]),
        ]
        queues = [nc.sync, nc.scalar, nc.gpsimd]
        for i, (dst, src) in enumerate(loads):
            queues[i % len(queues)].dma_start(dst[:], src[:, :, :])

        # stream of 80 chunks (block-major) through 4-bank PSUM tiles:
        # dots -> one exp sweep per tile -> per-block DVE partial reduces
        for j in range(NGRP):
            ps = pmm.tile([128, GRP, NW], F32, tag="mm", name=f"ps_{j}")
            for s in range(GRP):
                c = GRP * j + s               # global chunk index
                m, n = divmod(c, NCH)         # row block / class chunk
                q, r = divmod(m, NFP)
                for k2 in range(KC // 2):
                    nc.tensor.matmul(
                        ps[:, s, :],
                        fT[q][:, 2 * k2 : 2 * k2 + 2, r * 128 : (r + 1) * 128],
                        wT[n][:, 2 * k2 : 2 * k2 + 2, :],
                        start=(k2 == 0),
                        stop=(k2 == KC // 2 - 1),
                        perf_mode=mybir.MatmulPerfMode.DoubleRow,
                    )
            ed = esp.tile([128, GRP * NW], BF16, tag="ed", name=f"ed_{j}")
            nc.scalar.activation(
                ed[:], ps[:], AF.Exp,
                bias=nbias[:], scale=S / (F8S * F8S),
            )
            # slots grouped by row block (a tile spans at most 2 blocks);
            # tensor_scalar keeps the 2-byte fast path, the f32 row-sum goes
            # through the lane accumulator (cheap DVE_READ_ACCUMULATOR)
            b0 = (GRP * j) // NCH
            b1 = (GRP * j + GRP - 1) // NCH
            for bi, b in enumerate(range(b0, b1 + 1)):
                sl = [s for s in range(GRP) if (GRP * j + s) // NCH == b]
                s0, s1 = sl[0], sl[-1] + 1
                dm = dummy.tile([128, GRP * NW], BF16, tag="dm", name=f"dm_{j}_{bi}")
                nc.vector.tensor_scalar(
                    dm[:, 0 : (s1 - s0) * NW],
                    ed[:, s0 * NW : s1 * NW],
                    1.0, 0.0,
                    mybir.AluOpType.mult, mybir.AluOpType.add,
                    accum_out=parts_t[:, 2 * j + bi : 2 * j + bi + 1],
                )
            if j == NGRP // 2 - 1:
                nc.gpsimd.dma_start(parts_ext[:, 0:NGRP], parts_t[:, 0:NGRP])
        nc.sync.dma_start(parts_ext[:, NGRP : 2 * NGRP], parts_t[:, NGRP : 2 * NGRP])

    nc.finalize()
    return nc


def _get_graph():
    global _GRAPH
    if _GRAPH is None:
        _GRAPH = build_graph()
    return _GRAPH


def _to_kpn(xT):
    """[D, N] (d-major) -> [128, KC, N] with partition p = d % 128, k = d // 128."""
    Dd, N = xT.shape
    return np.ascontiguousarray(xT.reshape(KC, 128, N).transpose(1, 0, 2))


def make_in_maps(f, lab_word2vec, lab_pinds=None):
    f = np.asarray(f, dtype=np.float64)
    w = np.asarray(lab_word2vec, dtype=np.float64)
    fn = np.linalg.norm(f, axis=1)
    wn = np.linalg.norm(w, axis=1)
    fhatT = (F8S * (f / fn[:, None]).T).astype(np.float32)   # [D, B]
    ft_kpn = _to_kpn(fhatT).astype(float8_e4m3)              # [128, KC, B]
    ft_pieces = [
        np.ascontiguousarray(ft_kpn[:, :, q * NW : (q + 1) * NW]) for q in range(NFP)
    ]
    in_maps = []
    for i in range(NCORES):
        wpad = np.zeros((CSHP, D), dtype=np.float64)
        wsh = w[i * CSH : (i + 1) * CSH]
        wpad[:CSH] = wsh / wn[i * CSH : (i + 1) * CSH, None]
        wt_kpn = _to_kpn((F8S * wpad.T).astype(np.float32)).astype(float8_e4m3)
        m = {f"ft8_{q}": ft_pieces[q] for q in range(NFP)}
        for n in range(NCH):
            m[f"wt8_{n}"] = np.ascontiguousarray(wt_kpn[:, :, n * NW : (n + 1) * NW])
        in_maps.append(m)
    return in_maps


def host_finish(outs, f, lab_word2vec, lab_pinds, lengths):
    """outs: list of 8 dicts with sexp. Returns float32 loss."""
    f = np.asarray(f, dtype=np.float64)
    w = np.asarray(lab_word2vec, dtype=np.float64)
    pinds = np.asarray(lab_pinds, dtype=np.int64)
    lens = np.asarray(lengths, dtype=np.int64)

    # S_shift[b] = sum_c exp(30 cos - 30)
    # parts[:, 2j+bi] is the partial sum of exp over the slots of PSUM
    # tile j that fall in row block b0(j)+bi; fold them back per block.
    blk_of_col = np.zeros(2 * NGRP, dtype=np.int64)
    col_used = np.zeros(2 * NGRP, dtype=bool)
    for j in range(NGRP):
        b0 = (GRP * j) // NCH
        b1 = (GRP * j + GRP - 1) // NCH
        for bi, b in enumerate(range(b0, b1 + 1)):
            blk_of_col[2 * j + bi] = b
            col_used[2 * j + bi] = True
    s_shift = np.zeros(B, dtype=np.float64)
    for i in range(NCORES):
        pa = outs[i]["parts"].astype(np.float64)         # [128, 2*NGRP]
        per_block = np.zeros((128, NBLK), dtype=np.float64)
        for col in range(2 * NGRP):
            if col_used[col]:
                per_block[:, blk_of_col[col]] += pa[:, col]
        s_shift += per_block.T.reshape(B)                # b = m*128 + p
    # the 60 zero-pad classes per core contribute exp(-30) each (dot = 0)
    s_shift -= NCORES * (CSHP - CSH) * math.exp(-S)

    # exact positive-class cosines on host
    fn = np.linalg.norm(f, axis=1)                       # [B]
    wn = np.linalg.norm(w, axis=1)                       # [C]
    wsel = w[pinds]                                      # [B, LMAX, D]
    dots = np.einsum("bd,bld->bl", f, wsel)              # [B, LMAX]
    cos = dots / np.maximum(fn[:, None] * wn[pinds], 1e-8)

    cos_m, sin_m = math.cos(M_MARGIN), math.sin(M_MARGIN)
    th = math.cos(math.pi - M_MARGIN)
    mm = math.sin(math.pi - M_MARGIN) * M_MARGIN
    sine = np.sqrt(np.clip(1.0 - cos * cos, 0.0, 1.0))
    phi = cos * cos_m - sine * sin_m
    phi = np.where(cos > th, phi, cos - mm)

    mask = (np.arange(LMAX)[None, :] < lens[:, None])    # [B, LMAX] bool
    # dedup: a class replaced once in the denominator even if in 2 slots
    dup = np.zeros_like(mask)
    for j in range(1, LMAX):
        for j2 in range(j):
            dup[:, j] |= mask[:, j2] & (pinds[:, j2] == pinds[:, j])
    uniq = mask & ~dup
    corr = (uniq * (np.exp(S * phi - S) - np.exp(S * cos - S))).sum(axis=1)
    z = S + np.log(s_shift + corr)                       # logsumexp, [B]
    pos_sum = (mask * (S * phi)).sum(axis=1)
    L = lens.astype(np.float64)
    per_sample = (L * z - pos_sum) / (L * L)
    return np.float32(per_sample.mean())


def kernel(f, labels, lab_word2vec, lab_pinds, lengths):
    nc = _get_graph()
    in_maps = make_in_maps(f, lab_word2vec)
    res = run_bass_kernel_spmd(nc, in_maps, core_ids=list(range(NCORES)))
    return host_finish(res.results, f, lab_word2vec, lab_pinds, lengths)
